# revision 1
# baseline (speedup 1.0000x reference)
"""Trainium2 Bass kernel for nn_BinaryLinear (binarized linear layer).

Computes: out = sign(x) @ sign(W).T + bias
  x: [8192, 4096] f32, W: [4096, 4096] f32, bias: [4096] f32 -> out [8192, 4096] f32
  sign(v) = +1 if v >= 0 else -1

Sharding: 4x2 grid over 8 NeuronCores — batch split 4 ways (2048 rows each),
W rows (out_features) split 2 ways (2048 each). Each core computes a disjoint
[2048, 2048] output block; no collectives. This minimizes per-core input bytes
(32 MiB x-shard + 32 MiB W-shard).

Device-side algorithm (per core):
  1. Prep: DMA f32 tiles in natural layout, transpose 128x128 tiles on the
     TensorEngine (identity matmul), then binarize PSUM -> SBUF fp8 as +-0.5 in
     a single DVE op: (v >= 0 ? 1 : 0) - 0.5. Both operands end up [K-on-
     partitions, rows-on-free] fp8, fully resident in SBUF.
  2. Matmul: standard K-accumulated PE matmuls, psum = (sum of +-0.25 terms)
     = exact_integer_result / 4. fp8 +-0.5 operands are exact, and the fp32
     PSUM accumulation of quarter-integers bounded by 1024 is exact.
  3. Epilogue: ACT copy with scale=4 (psum*4 -> exact integers), DVE add of the
     bias row (pre-replicated across 128 partitions on host), DMA out.
"""

import math
import os

import numpy as np

import concourse.bacc as bacc
import concourse.mybir as mybir
import concourse.tile as tile
from concourse.alu_op_type import AluOpType
from concourse.bass_utils import run_bass_kernel_spmd
from concourse.masks import make_identity

P = 128
N_CORES = 8
M_SPLIT = 4  # batch split
N_SPLIT = 2  # out_features split

# Full-problem shapes (hardcoded per harness contract)
BATCH = 8192
IN_FEATURES = 4096
OUT_FEATURES = 4096

F32 = mybir.dt.float32
FP8 = mybir.dt.float8e4

SUPER = 512  # rows per prep "super" == matmul o-panel width (one PSUM bank)


def build_nc(M, K, N, n_cores=N_CORES, double_row=True):
    """Build the per-core kernel: x_shard [M, K], w_shard [N, K],
    bias_rep [P, N] -> out_shard [M, N]."""
    assert M % SUPER == 0 and N % SUPER == 0 and K % P == 0
    KT = K // P  # contraction tiles
    M_SUPERS = M // SUPER
    N_SUPERS = N // SUPER
    RB = SUPER // P  # row-blocks per super (4)
    I_CHUNK = min(1024, K)  # staged i-columns per DMA
    ICT = I_CHUNK // P
    if double_row:
        assert KT % 2 == 0

    nc = bacc.Bacc(
        "TRN2", target_bir_lowering=False, debug=False, num_devices=n_cores
    )
    x_in = nc.dram_tensor("x_shard", [M, K], F32, kind="ExternalInput").ap()
    w_in = nc.dram_tensor("w_shard", [N, K], F32, kind="ExternalInput").ap()
    b_in = nc.dram_tensor("bias_rep", [P, N], F32, kind="ExternalInput").ap()
    out = nc.dram_tensor("out_shard", [M, N], F32, kind="ExternalOutput").ap()

    with tile.TileContext(nc) as tc:
        with (
            tc.tile_pool(name="const", bufs=1) as const,
            tc.tile_pool(name="resid", bufs=1) as resid,
            tc.tile_pool(name="stage", bufs=2) as stage_pool,
            tc.tile_pool(name="tps", bufs=2, space="PSUM") as tps_pool,
            tc.tile_pool(name="mm", bufs=2, space="PSUM") as mm_pool,
            tc.tile_pool(name="outp", bufs=3) as out_pool,
        ):
            identity = const.tile([P, P], F32, name="identity", tag="identity")
            make_identity(nc, identity)
            bias_sb = const.tile([P, N], F32, name="bias_sb", tag="bias_sb")
            nc.sync.dma_start(bias_sb, b_in)

            xT = [
                resid.tile([P, KT, SUPER], FP8, name=f"xT{s}", tag=f"xT{s}")
                for s in range(M_SUPERS)
            ]
            wT = [
                resid.tile([P, KT, SUPER], FP8, name=f"wT{s}", tag=f"wT{s}")
                for s in range(N_SUPERS)
            ]

            def prep_super(src_ap, dstT, s):
                """Transpose+binarize rows [s*SUPER, (s+1)*SUPER) of src into
                dstT [P, KT, SUPER] fp8 (+-0.5)."""
                for ic in range(K // I_CHUNK):
                    stages = []
                    for j in range(RB):
                        r0 = s * SUPER + j * P
                        st = stage_pool.tile(
                            [P, I_CHUNK], F32, name=f"stage{j}", tag=f"stage{j}"
                        )
                        nc.sync.dma_start(
                            st, src_ap[r0 : r0 + P, ic * I_CHUNK : (ic + 1) * I_CHUNK]
                        )
                        stages.append(st)
                    for ktl in range(ICT):
                        ps = tps_pool.tile([P, SUPER], F32, name="tps", tag="tps")
                        for j in range(RB):
                            nc.tensor.transpose(
                                ps[:, j * P : (j + 1) * P],
                                stages[j][:, ktl * P : (ktl + 1) * P],
                                identity,
                            )
                        kt = ic * ICT + ktl
                        # (v >= 0 ? 1.0 : 0.0) - 0.5  ->  +-0.5 in fp8
                        nc.vector.tensor_scalar(
                            out=dstT[:, kt, :],
                            in0=ps,
                            scalar1=0.0,
                            scalar2=0.5,
                            op0=AluOpType.is_ge,
                            op1=AluOpType.subtract,
                        )

            def main_block(ms, os_):
                """Compute out rows [ms*SUPER ...) x cols [os_*SUPER ...)."""
                for mt in range(RB):
                    psum = mm_pool.tile([P, SUPER], F32, name="mmps", tag="mmps")
                    if double_row:
                        for kt in range(0, KT, 2):
                            nc.tensor.matmul(
                                psum,
                                lhsT=xT[ms][:, kt : kt + 2, mt * P : (mt + 1) * P],
                                rhs=wT[os_][:, kt : kt + 2, :],
                                start=(kt == 0),
                                stop=(kt == KT - 2),
                                perf_mode=mybir.MatmulPerfMode.DoubleRow,
                            )
                    else:
                        for kt in range(KT):
                            nc.tensor.matmul(
                                psum,
                                lhsT=xT[ms][:, kt, mt * P : (mt + 1) * P],
                                rhs=wT[os_][:, kt, :],
                                start=(kt == 0),
                                stop=(kt == KT - 1),
                            )
                    ob = out_pool.tile([P, SUPER], F32, name="ob", tag="ob")
                    # psum holds exact_int/4; scale back to exact integers
                    nc.scalar.activation(
                        ob, psum, mybir.ActivationFunctionType.Copy, scale=4.0
                    )
                    nc.vector.tensor_tensor(
                        ob,
                        ob,
                        bias_sb[:, os_ * SUPER : (os_ + 1) * SUPER],
                        AluOpType.add,
                    )
                    r0 = ms * SUPER + mt * P
                    nc.sync.dma_start(
                        out[r0 : r0 + P, os_ * SUPER : (os_ + 1) * SUPER], ob
                    )

            # Emission order interleaves prep and matmul so PE never starves:
            # x-super 0, then each w-super immediately followed by the main
            # blocks it unlocks; remaining x-supers each unlock a row of blocks.
            prep_super(x_in, xT[0], 0)
            for os_ in range(N_SUPERS):
                prep_super(w_in, wT[os_], os_)
                main_block(0, os_)
            for ms in range(1, M_SUPERS):
                prep_super(x_in, xT[ms], ms)
                for os_ in range(N_SUPERS):
                    main_block(ms, os_)

    nc.compile()
    return nc


_NC_CACHE = {}


def _get_nc(M, K, N, double_row=True):
    key = (M, K, N, double_row)
    if key not in _NC_CACHE:
        _NC_CACHE[key] = build_nc(M, K, N, double_row=double_row)
    return _NC_CACHE[key]


LAST_RESULTS = None


def make_in_maps(x, weight, bias):
    MS = x.shape[0] // M_SPLIT
    NS = weight.shape[0] // N_SPLIT
    in_maps = []
    for c in range(N_CORES):
        mi, ni = divmod(c, N_SPLIT)
        in_maps.append(
            {
                "x_shard": np.ascontiguousarray(x[mi * MS : (mi + 1) * MS]),
                "w_shard": np.ascontiguousarray(weight[ni * NS : (ni + 1) * NS]),
                "bias_rep": np.ascontiguousarray(
                    np.broadcast_to(bias[None, ni * NS : (ni + 1) * NS], (P, NS))
                ),
            }
        )
    return in_maps


def kernel(x, weight, bias):
    global LAST_RESULTS
    x = np.ascontiguousarray(np.asarray(x, dtype=np.float32))
    weight = np.ascontiguousarray(np.asarray(weight, dtype=np.float32))
    bias = np.ascontiguousarray(np.asarray(bias, dtype=np.float32))
    B, K = x.shape
    O = weight.shape[0]
    assert B % M_SPLIT == 0 and O % N_SPLIT == 0

    double_row = os.environ.get("BINLIN_DOUBLE_ROW", "1") == "1"
    nc = _get_nc(B // M_SPLIT, K, O // N_SPLIT, double_row=double_row)
    in_maps = make_in_maps(x, weight, bias)

    res = run_bass_kernel_spmd(nc, in_maps, core_ids=list(range(N_CORES)))
    LAST_RESULTS = res

    MS = B // M_SPLIT
    NS = O // N_SPLIT
    out = np.empty((B, O), dtype=np.float32)
    for c in range(N_CORES):
        mi, ni = divmod(c, N_SPLIT)
        out[mi * MS : (mi + 1) * MS, ni * NS : (ni + 1) * NS] = res.results[c][
            "out_shard"
        ]
    return out
